# revision 5
# baseline (speedup 1.0000x reference)
# Trainium2 Bass kernel for nn_LogitsNew (dense_mlp).
#
#   u = gelu(x @ W_proj + b_proj)                       [B, D]
#   logits = (u @ W_u)[:, None, :] + ee @ W_e           [B, N, C]
#
# Sharding: data-parallel over batch B across 8 cores (4 batches/core).
#
# v3 design (93.7us baseline -> 65.8us v2 -> this):
#   - fp16 end to end (fp32 PSUM accumulation), host-side packing:
#     ee pre-transposed to PE-stationary layout, x pre-transposed +
#     b_proj as a per-partition column -> no identity matrix, no PE
#     transposes anywhere.
#   - z computed TRANSPOSED (zT[d,b] via W_proj-chunk stationary), so
#     gelu(zT)+bias-column directly yields uT for the y matmul.
#   - DMA: few large dma_starts (the ~2us fixed completion cost per
#     dma_start dominated v2), with small latency-optimized first chunks
#     (ee m0 / W_e k0,k1) so the PE starts ~9us in. Loads split across
#     the ACT + SP rings; y_row collapse on the gpsimd (SWDGE) ring.
#   - 8 warmup matmuls on a memset tile bring the PE out of the cold
#     1.2GHz HAM state during the initial DMA wait.
#   - drains (PSUM->SBUF fp16) on scalar, +y adds on vector, 3 grouped
#     stores on SP; output in device layout [p, mt, c], unpacked on host.

import sys

if "/opt/trn_rl_repo" not in sys.path:
    sys.path.insert(0, "/opt/trn_rl_repo")

import numpy as np

import concourse.bass as bass
import concourse.mybir as mybir
import concourse.tile as tile
from concourse import bacc
from concourse.bass_utils import run_bass_kernel_spmd

P = 128
B, N, D, C = 32, 256, 1024, 1024
NCORES = 8
BPC = B // NCORES          # batches per core
KT = D // P                # 8 k-tiles over the contraction dim
FD = 512                   # matmul moving free dim (one PSUM bank of fp32)
NT = N // P                # 2 n-tiles per batch
MT = BPC * NT              # 8 m-tiles per core
XBW = KT * BPC + KT        # xb packed width: xT (32) + b column (8)

F32 = mybir.dt.float32
F16 = mybir.dt.float16
GELU = mybir.ActivationFunctionType.Gelu

_CACHE = {}


def _build():
    if "nc" in _CACHE:
        return _CACHE["nc"]

    nc = bacc.Bacc("TRN2", target_bir_lowering=False, debug=False, num_devices=NCORES)

    # host-packed inputs (all fp16):
    #   ee_t[p, m, k, f] = ee[b, nh*128+f, k*128+p], m = b*NT+nh
    #   we_t/wu_t/wp_t[p, k, c] = W[k*128+p, c]
    #   xb[p, k*BPC+b] = x[b, k*128+p]; xb[p, 32+k] = b_proj[k*128+p]
    ee_t = nc.dram_tensor("ee_t", [P, MT, KT, P], F16, kind="ExternalInput").ap()
    we_t = nc.dram_tensor("we_t", [P, KT, C], F16, kind="ExternalInput").ap()
    wu_t = nc.dram_tensor("wu_t", [P, KT, C], F16, kind="ExternalInput").ap()
    wp_t = nc.dram_tensor("wp_t", [P, KT, C], F16, kind="ExternalInput").ap()
    xb = nc.dram_tensor("xb", [P, XBW], F16, kind="ExternalInput").ap()
    out = nc.dram_tensor("out_t", [P, MT, C], F16, kind="ExternalOutput").ap()

    with tile.TileContext(nc) as tc:
        with (
            tc.tile_pool(name="const", bufs=1) as cpool,
            tc.tile_pool(name="weights", bufs=1) as wpool,
            tc.tile_pool(name="outs", bufs=1) as outpool,
            tc.tile_pool(name="warm_ps", bufs=1, space="PSUM") as warm_ps,
            tc.tile_pool(name="zy_ps", bufs=2, space="PSUM") as zy_ps,
            tc.tile_pool(name="mm_ps", bufs=4, space="PSUM") as mm_ps,
        ):
            xb_sb = cpool.tile([P, XBW], F16)
            we16 = wpool.tile([P, KT, C], F16)
            wp16 = wpool.tile([P, KT, C], F16)
            wu16 = wpool.tile([P, KT, C], F16)
            ee_sb = cpool.tile([P, MT, KT, P], F16)
            o_all = outpool.tile([P, MT, C], F16)

            # warmup source (gpsimd memset, ready before any DMA lands)
            wsrc = cpool.tile([1, P + FD], F16)
            nc.gpsimd.memset(wsrc, 0.0)

            # ---- ACT ring: latency-critical first chunks ----
            nc.scalar.dma_start(xb_sb, xb)
            nc.scalar.dma_start(ee_sb[:, 0:1], ee_t[:, 0:1])
            nc.scalar.dma_start(we16[:, 0:1], we_t[:, 0:1])
            nc.scalar.dma_start(we16[:, 1:2], we_t[:, 1:2])
            nc.scalar.dma_start(ee_sb[:, 1:2], ee_t[:, 1:2])
            nc.scalar.dma_start(we16[:, 2:4], we_t[:, 2:4])
            nc.scalar.dma_start(ee_sb[:, 2:8], ee_t[:, 2:8])

            # ---- SP ring: bulk weights ----
            nc.sync.dma_start(we16[:, 4:8], we_t[:, 4:8])
            nc.sync.dma_start(wp16, wp_t)
            nc.sync.dma_start(wu16, wu_t)

            # ---- PE warmup: 8 junk matmuls to lift the HAM clock gate ----
            for _ in range(8):
                wp_ps = warm_ps.tile([P, FD], F32, tag="warm")
                nc.tensor.matmul(wp_ps, wsrc[:1, :P], wsrc[:1, P:], start=True, stop=True)

            out_ready = []

            def main_mtile(mt):
                mps = [
                    mm_ps.tile([P, FD], F32, tag="mm", name=f"mm_{mt}_{ch}")
                    for ch in range(2)
                ]
                for k in range(KT):
                    for ch in range(2):
                        nc.tensor.matmul(
                            mps[ch],
                            ee_sb[:, mt, k, :],
                            we16[:, k, ch * FD : (ch + 1) * FD],
                            start=(k == 0),
                            stop=(k == KT - 1),
                        )
                nc.scalar.copy(o_all[:, mt, 0:FD], mps[0])
                nc.scalar.copy(o_all[:, mt, FD:C], mps[1])

            main_mtile(0)
            main_mtile(1)
            main_mtile(2)

            # ---- zT = (x @ W_proj).T ; uT = gelu(zT + b) ----
            # zT[p, kc, b] = z[b, kc*128+p]; stationary = W_proj chunk.
            ztp = zy_ps.tile([P, KT * BPC], F32, tag="zy", name="zt")
            for kc in range(KT):
                for kd in range(KT):
                    nc.tensor.matmul(
                        ztp[:, kc * BPC : (kc + 1) * BPC],
                        wp16[:, kd, kc * P : (kc + 1) * P],
                        xb_sb[:, kd * BPC : (kd + 1) * BPC],
                        start=(kd == 0),
                        stop=(kd == KT - 1),
                    )
            uT = cpool.tile([P, KT, BPC], F16)
            for kc in range(KT):
                nc.scalar.activation(
                    uT[:, kc, :],
                    ztp[:, kc * BPC : (kc + 1) * BPC],
                    GELU,
                    bias=xb_sb[:, KT * BPC + kc : KT * BPC + kc + 1],
                )

            main_mtile(3)

            # ---- y = u @ W_u, broadcast across partitions ----
            yp = [zy_ps.tile([P, FD], F32, tag="zy", name=f"y_{ch}") for ch in range(2)]
            for k in range(KT):
                for ch in range(2):
                    nc.tensor.matmul(
                        yp[ch][:BPC],
                        uT[:, k, :],
                        wu16[:, k, ch * FD : (ch + 1) * FD],
                        start=(k == 0),
                        stop=(k == KT - 1),
                    )
            y16 = cpool.tile([BPC, C], F16)
            nc.vector.tensor_copy(y16[:, 0:FD], yp[0][:BPC])
            nc.vector.tensor_copy(y16[:, FD:C], yp[1][:BPC])
            y_row = cpool.tile([1, BPC, C], F16)
            nc.gpsimd.dma_start(y_row, y16)
            ybc = cpool.tile([P, BPC, C], F16)
            for b2 in range(BPC):
                nc.gpsimd.partition_broadcast(ybc[:, b2, :], y_row[:1, b2, :])

            for mt in range(4, MT):
                main_mtile(mt)

            # ---- epilogue: add broadcast y (vector), grouped stores (SP) ----
            for mt in range(MT):
                b = mt // NT
                nc.vector.tensor_add(o_all[:, mt, :], o_all[:, mt, :], ybc[:, b, :])
                if mt == 3:
                    nc.sync.dma_start(out[:, 0:4], o_all[:, 0:4])
                elif mt == 6:
                    nc.sync.dma_start(out[:, 4:7], o_all[:, 4:7])
                elif mt == 7:
                    nc.sync.dma_start(out[:, 7:8], o_all[:, 7:8])

    nc.compile()
    _CACHE["nc"] = nc
    return nc


def _pack(inputs):
    """Host-side dtype conversion + layout packing (no arithmetic)."""
    x = np.asarray(inputs["encoded_utterance"], np.float32)
    ee = np.asarray(inputs["element_embeddings"], np.float32)
    w = np.asarray(inputs["weight_matrix"], np.float32)
    wp = np.asarray(inputs["W_proj"], np.float32)
    b = np.asarray(inputs["b_proj"], np.float32).reshape(D)

    # [2D, C] -> [p, k, c]
    wu_t = np.ascontiguousarray(
        w[:D].reshape(KT, P, C).transpose(1, 0, 2).astype(np.float16)
    )
    we_t = np.ascontiguousarray(
        w[D:].reshape(KT, P, C).transpose(1, 0, 2).astype(np.float16)
    )
    wp_t = np.ascontiguousarray(
        wp.reshape(KT, P, C).transpose(1, 0, 2).astype(np.float16)
    )
    bcol = b.reshape(KT, P).T.astype(np.float16)  # [p, k]

    # ee[b, n, d] -> [p(d), m, k(d), f(n)], m = b*NT+nh
    ee16 = ee.astype(np.float16)
    x16 = x.astype(np.float16)
    in_maps = []
    for i in range(NCORES):
        bs = slice(i * BPC, (i + 1) * BPC)
        ee_t = np.ascontiguousarray(
            ee16[bs].reshape(MT, P, KT, P).transpose(3, 0, 2, 1)
        )
        xbm = np.empty((P, XBW), np.float16)
        xbm[:, : KT * BPC] = (
            x16[bs].T.reshape(KT, P, BPC).transpose(1, 0, 2).reshape(P, KT * BPC)
        )
        xbm[:, KT * BPC :] = bcol
        in_maps.append(
            {"ee_t": ee_t, "we_t": we_t, "wu_t": wu_t, "wp_t": wp_t, "xb": xbm}
        )
    return in_maps


def run(inputs, trace=False, **kwargs):
    nc = _build()
    in_maps = _pack(inputs)
    res = run_bass_kernel_spmd(
        nc, in_maps, core_ids=list(range(NCORES)), trace=trace, **kwargs
    )
    # out_t[p, m, c] -> logits[b, nh*128+p, c]
    outs = []
    for r in res.results:
        o = r["out_t"].astype(np.float32)  # [P, MT, C]
        outs.append(o.transpose(1, 0, 2).reshape(BPC, N, C))
    full = np.concatenate(outs, axis=0)
    return full, res


def kernel(**inputs) -> np.ndarray:
    return run(inputs, trace=False)[0]


# revision 6
# speedup vs baseline: 1.2778x; 1.2778x over previous
# Trainium2 Bass kernel for nn_LogitsNew (dense_mlp).
#
#   u = gelu(x @ W_proj + b_proj)                       [B, D]
#   logits = (u @ W_u)[:, None, :] + ee @ W_e           [B, N, C]
#
# Sharding: data-parallel over batch B across 8 cores (4 batches/core).
#
# v3 design (93.7us baseline -> 65.8us v2 -> this):
#   - fp16 end to end (fp32 PSUM accumulation), host-side packing:
#     ee pre-transposed to PE-stationary layout, x pre-transposed +
#     b_proj as a per-partition column -> no identity matrix, no PE
#     transposes anywhere.
#   - z computed TRANSPOSED (zT[d,b] via W_proj-chunk stationary), so
#     gelu(zT)+bias-column directly yields uT for the y matmul.
#   - DMA: few large dma_starts (the ~2us fixed completion cost per
#     dma_start dominated v2), with small latency-optimized first chunks
#     (ee m0 / W_e k0,k1) so the PE starts ~9us in. Loads split across
#     the ACT + SP rings; y_row collapse on the gpsimd (SWDGE) ring.
#   - 8 warmup matmuls on a memset tile bring the PE out of the cold
#     1.2GHz HAM state during the initial DMA wait.
#   - drains (PSUM->SBUF fp16) on scalar, +y adds on vector, 3 grouped
#     stores on SP; output in device layout [p, mt, c], unpacked on host.

import sys

if "/opt/trn_rl_repo" not in sys.path:
    sys.path.insert(0, "/opt/trn_rl_repo")

import numpy as np

import concourse.bass as bass
import concourse.mybir as mybir
import concourse.tile as tile
from concourse import bacc
from concourse.bass_utils import run_bass_kernel_spmd

P = 128
B, N, D, C = 32, 256, 1024, 1024
NCORES = 8
BPC = B // NCORES          # batches per core
KT = D // P                # 8 k-tiles over the contraction dim
FD = 512                   # matmul moving free dim (one PSUM bank of fp32)
NT = N // P                # 2 n-tiles per batch
MT = BPC * NT              # 8 m-tiles per core
XBW = KT * BPC + KT        # xb packed width: xT (32) + b column (8)

F32 = mybir.dt.float32
F16 = mybir.dt.float16
GELU = mybir.ActivationFunctionType.Gelu

_CACHE = {}


def _build():
    if "nc" in _CACHE:
        return _CACHE["nc"]

    nc = bacc.Bacc("TRN2", target_bir_lowering=False, debug=False, num_devices=NCORES)

    # host-packed inputs (all fp16):
    #   ee_t[p, m, k, f] = ee[b, nh*128+f, k*128+p], m = b*NT+nh
    #   we_t/wu_t/wp_t[p, k, c] = W[k*128+p, c]
    #   xb[p, k*BPC+b] = x[b, k*128+p]; xb[p, 32+k] = b_proj[k*128+p]
    ee_t = nc.dram_tensor("ee_t", [P, MT, KT, P], F16, kind="ExternalInput").ap()
    we_t = nc.dram_tensor("we_t", [P, KT, C], F16, kind="ExternalInput").ap()
    wu_t = nc.dram_tensor("wu_t", [P, KT, C], F16, kind="ExternalInput").ap()
    wp_t = nc.dram_tensor("wp_t", [P, KT, C], F16, kind="ExternalInput").ap()
    xb = nc.dram_tensor("xb", [P, XBW], F16, kind="ExternalInput").ap()
    out = nc.dram_tensor("out_t", [P, MT, C], F16, kind="ExternalOutput").ap()

    with tile.TileContext(nc) as tc:
        with (
            tc.tile_pool(name="const", bufs=1) as cpool,
            tc.tile_pool(name="weights", bufs=1) as wpool,
            tc.tile_pool(name="outs", bufs=1) as outpool,
            tc.tile_pool(name="warm_ps", bufs=1, space="PSUM") as warm_ps,
            tc.tile_pool(name="zy_ps", bufs=2, space="PSUM") as zy_ps,
            tc.tile_pool(name="mm_ps", bufs=4, space="PSUM") as mm_ps,
        ):
            xb_sb = cpool.tile([P, XBW], F16)
            we16 = wpool.tile([P, KT, C], F16)
            wp16 = wpool.tile([P, KT, C], F16)
            wu16 = wpool.tile([P, KT, C], F16)
            ee_sb = cpool.tile([P, MT, KT, P], F16)
            o_all = outpool.tile([P, MT, C], F16)

            # warmup source (gpsimd memset, ready before any DMA lands)
            wsrc = cpool.tile([1, P + FD], F16)
            nc.gpsimd.memset(wsrc, 0.0)

            # ---- ALL inputs on the ACT ring in PE-consumption order.
            # (The 16 DMA queues round-robin between rings per descriptor,
            # so a second ring of bulk loads would starve these.)
            nc.scalar.dma_start(xb_sb, xb)
            nc.scalar.dma_start(ee_sb[:, 0:1], ee_t[:, 0:1])
            nc.scalar.dma_start(ee_sb[:, 1:2], ee_t[:, 1:2])
            for j in range(4):
                nc.scalar.dma_start(we16[:, 2 * j : 2 * j + 2], we_t[:, 2 * j : 2 * j + 2])
            nc.scalar.dma_start(ee_sb[:, 2:4], ee_t[:, 2:4])
            nc.scalar.dma_start(wp16, wp_t)
            nc.scalar.dma_start(wu16, wu_t)
            nc.scalar.dma_start(ee_sb[:, 4:8], ee_t[:, 4:8])

            # ---- PE warmup: 8 junk matmuls to lift the HAM clock gate ----
            for _ in range(8):
                wp_ps = warm_ps.tile([P, FD], F32, tag="warm")
                nc.tensor.matmul(wp_ps, wsrc[:1, :P], wsrc[:1, P:], start=True, stop=True)

            out_ready = []

            def main_mtile(mt):
                mps = [
                    mm_ps.tile([P, FD], F32, tag="mm", name=f"mm_{mt}_{ch}")
                    for ch in range(2)
                ]
                for k in range(KT):
                    for ch in range(2):
                        nc.tensor.matmul(
                            mps[ch],
                            ee_sb[:, mt, k, :],
                            we16[:, k, ch * FD : (ch + 1) * FD],
                            start=(k == 0),
                            stop=(k == KT - 1),
                        )
                nc.scalar.copy(o_all[:, mt, 0:FD], mps[0])
                nc.scalar.copy(o_all[:, mt, FD:C], mps[1])

            main_mtile(0)
            main_mtile(1)
            main_mtile(2)

            # ---- zT = (x @ W_proj).T ; uT = gelu(zT + b) ----
            # zT[p, kc, b] = z[b, kc*128+p]; stationary = W_proj chunk.
            ztp = zy_ps.tile([P, KT * BPC], F32, tag="zy", name="zt")
            for kc in range(KT):
                for kd in range(KT):
                    nc.tensor.matmul(
                        ztp[:, kc * BPC : (kc + 1) * BPC],
                        wp16[:, kd, kc * P : (kc + 1) * P],
                        xb_sb[:, kd * BPC : (kd + 1) * BPC],
                        start=(kd == 0),
                        stop=(kd == KT - 1),
                    )
            uT = cpool.tile([P, KT, BPC], F16)
            for kc in range(KT):
                nc.scalar.activation(
                    uT[:, kc, :],
                    ztp[:, kc * BPC : (kc + 1) * BPC],
                    GELU,
                    bias=xb_sb[:, KT * BPC + kc : KT * BPC + kc + 1],
                )

            main_mtile(3)

            # ---- y = u @ W_u, broadcast across partitions ----
            yp = [zy_ps.tile([P, FD], F32, tag="zy", name=f"y_{ch}") for ch in range(2)]
            for k in range(KT):
                for ch in range(2):
                    nc.tensor.matmul(
                        yp[ch][:BPC],
                        uT[:, k, :],
                        wu16[:, k, ch * FD : (ch + 1) * FD],
                        start=(k == 0),
                        stop=(k == KT - 1),
                    )
            y16 = cpool.tile([BPC, C], F16)
            nc.vector.tensor_copy(y16[:, 0:FD], yp[0][:BPC])
            nc.vector.tensor_copy(y16[:, FD:C], yp[1][:BPC])
            y_row = cpool.tile([1, BPC, C], F16)
            nc.gpsimd.dma_start(y_row, y16)
            ybc = cpool.tile([P, BPC, C], F16)
            for b2 in range(BPC):
                nc.gpsimd.partition_broadcast(ybc[:, b2, :], y_row[:1, b2, :])

            for mt in range(4, MT):
                main_mtile(mt)

            # ---- epilogue: add broadcast y (vector), grouped stores (SP) ----
            for mt in range(MT):
                b = mt // NT
                nc.vector.tensor_add(o_all[:, mt, :], o_all[:, mt, :], ybc[:, b, :])
                if mt == 3:
                    nc.sync.dma_start(out[:, 0:4], o_all[:, 0:4])
                elif mt == 6:
                    nc.sync.dma_start(out[:, 4:7], o_all[:, 4:7])
                elif mt == 7:
                    nc.sync.dma_start(out[:, 7:8], o_all[:, 7:8])

    nc.compile()
    _CACHE["nc"] = nc
    return nc


def _pack(inputs):
    """Host-side dtype conversion + layout packing (no arithmetic)."""
    x = np.asarray(inputs["encoded_utterance"], np.float32)
    ee = np.asarray(inputs["element_embeddings"], np.float32)
    w = np.asarray(inputs["weight_matrix"], np.float32)
    wp = np.asarray(inputs["W_proj"], np.float32)
    b = np.asarray(inputs["b_proj"], np.float32).reshape(D)

    # [2D, C] -> [p, k, c]
    wu_t = np.ascontiguousarray(
        w[:D].reshape(KT, P, C).transpose(1, 0, 2).astype(np.float16)
    )
    we_t = np.ascontiguousarray(
        w[D:].reshape(KT, P, C).transpose(1, 0, 2).astype(np.float16)
    )
    wp_t = np.ascontiguousarray(
        wp.reshape(KT, P, C).transpose(1, 0, 2).astype(np.float16)
    )
    bcol = b.reshape(KT, P).T.astype(np.float16)  # [p, k]

    # ee[b, n, d] -> [p(d), m, k(d), f(n)], m = b*NT+nh
    ee16 = ee.astype(np.float16)
    x16 = x.astype(np.float16)
    in_maps = []
    for i in range(NCORES):
        bs = slice(i * BPC, (i + 1) * BPC)
        ee_t = np.ascontiguousarray(
            ee16[bs].reshape(MT, P, KT, P).transpose(3, 0, 2, 1)
        )
        xbm = np.empty((P, XBW), np.float16)
        xbm[:, : KT * BPC] = (
            x16[bs].T.reshape(KT, P, BPC).transpose(1, 0, 2).reshape(P, KT * BPC)
        )
        xbm[:, KT * BPC :] = bcol
        in_maps.append(
            {"ee_t": ee_t, "we_t": we_t, "wu_t": wu_t, "wp_t": wp_t, "xb": xbm}
        )
    return in_maps


def run(inputs, trace=False, **kwargs):
    nc = _build()
    in_maps = _pack(inputs)
    res = run_bass_kernel_spmd(
        nc, in_maps, core_ids=list(range(NCORES)), trace=trace, **kwargs
    )
    # out_t[p, m, c] -> logits[b, nh*128+p, c]
    outs = []
    for r in res.results:
        o = r["out_t"].astype(np.float32)  # [P, MT, C]
        outs.append(o.transpose(1, 0, 2).reshape(BPC, N, C))
    full = np.concatenate(outs, axis=0)
    return full, res


def kernel(**inputs) -> np.ndarray:
    return run(inputs, trace=False)[0]
